# revision 60
# baseline (speedup 1.0000x reference)
"""Sharded causal multi-head attention for 8 Trainium2 NeuronCores.

kernel(**inputs) takes the FULL inputs (Q, K, V, mask, Wq, bq, Wk, bk,
Wv, bv, Wo, bo) and returns the FULL [2, 2048, 1024] float32 output.

Sharding (data + head/tensor parallel): core c = 4*b + g handles batch
b in {0,1} and head-group g in {0..3} (4 heads, 256 dims). W_q/W_k/W_v
are column-parallel, W_o row-parallel; the host sums the 4 per-batch
row-parallel partials and adds bo.

Per-core program (Bass/Tile, fp16 compute, fp32 PSUM accumulation),
software-pipelined per S-quarter so projections, attention, and the
output projection all interleave on the PE:

  for quarter in 0..3:
      project q/k/v for this quarter       (PE + DVE bias-evac)
      output-project q-block quarter-1     (PE slack + DVE copy)
      flash attention for q-block quarter  (PE/ACT/DVE pipeline)
  output-project q-block 3

Attention runs in scores^T layout [k, q] per head-pair pass so the exp
covers two heads' score tiles with one ScalarE instruction (PSUM tiles
span 2 banks), amortizing the fixed PSUM/SBUF access overhead. Fully
masked 128-col sub-blocks of diagonal tiles are never computed: score
matmuls, exp, and attn@V all start at the first live column. Causal
masking multiplies a single [128,128] lower-tri tile on the diagonal
block only. Rowsums ride along in the attn@V matmul via a 64-wide ones
block in the stationary operand; odd heads flip [dims|ones] so their
output lands on partitions 64:128 directly (no staging DMA), and the
1/rowsum lives on DVE (reciprocal + cross-partition-base multiply,
which hardware permits when one operand is in PSUM).

Inputs are host-relayouted so every DMA descriptor is an 8KB
contiguous row; x-input DMAs issue from the Sync queue, consts and
output DMAs from the GpSimd queue.
"""

import json
import sys

for _p in ("/opt/trn_rl_repo", "/opt/trn_rl_repo/concourse"):
    if _p not in sys.path:
        sys.path.insert(0, _p)

import numpy as np

import bass_rust
import concourse.bass as bass
import concourse.mybir as mybir
import concourse.tile as tile
from concourse import bass_utils
from concourse.vector_clock import ScopedClock

F32 = mybir.dt.float32
F16 = mybir.dt.float16
S = 2048
D = 1024
HG = 256  # head-group dims (4 heads x 64)
KC = D // 128
QB = 512  # S-quarter

# --------------------------------------------------------------------------
# Environment patches: this container's walrus accepts only ONE sync-wait
# command per instruction, but Tile emits several (and its epilogue drain
# carries one per outstanding proc sem). Split extras onto single-wait NoOps.
# --------------------------------------------------------------------------

_patched = False


def _drain_and_barrier_split(self, tick_clock, wait_clock):
    nc = self.nc
    probe = nc.sync.nop()
    wait_clock.add_sem_waits(probe.ins, ScopedClock({None: tick_clock.global_clock}))
    si = probe.ins.sync_info
    waits = list(si.on_wait) if si is not None and si.on_wait else []
    if len(waits) > 1:
        si.on_wait = [waits[0]]
        for w in waits[1:]:
            nop = nc.sync.nop()
            nop.ins.sync_info = bass_rust.SyncInfo(on_wait=[w], on_update=[])
    nc.sync.drain()
    nc.all_engine_barrier()
    assert self.sems is not None
    popped = nc._tile_sem_poison_stack.pop()
    assert popped is self._sem_poison
    # Skip the per-sem teardown clears + trailing barrier (~7us): the program
    # prologue resets the whole kernel sem range before use, so a fresh
    # execution of this NEFF never observes the dirty values. Keep the pool
    # bookkeeping so tile-system invariants hold during build.
    nc._state.prepend_free_semaphores(
        [s.num for s in self.sems.allocated().values()]
    )
    for poison_set in nc._tile_sem_poison_stack:
        poison_set.update(s.num for s in self.sems.allocated().values())


def _split_waits_json(raw):
    j = json.loads(raw)
    changed = False
    for f in j.get("functions", []):
        for bb in f.get("blocks", []):
            out = []
            for inst in bb.get("instructions", []):
                si = inst.get("sync_info")
                waits = (si or {}).get("on_wait") or []
                if len(waits) > 1:
                    for k, w in enumerate(waits[:-1]):
                        nop = {
                            "engine": inst["engine"],
                            "ins": [],
                            "name": f"{inst['name']}-ws{k}",
                            "opcode": "NoOp",
                            "outs": [],
                            "sync_info": {"on_update": [], "on_wait": [w]},
                        }
                        if "debug" in inst:
                            nop["debug"] = inst["debug"]
                        out.append(nop)
                    si["on_wait"] = [waits[-1]]
                    changed = True
                out.append(inst)
            if changed:
                bb["instructions"] = out
    return json.dumps(j).encode() if changed else raw


def _apply_patches():
    global _patched
    if _patched:
        return
    tile.TileContext._drain_and_barrier = _drain_and_barrier_split
    orig_to_json = bass.Bass.to_json_bytes
    bass.Bass.to_json_bytes = lambda self: _split_waits_json(orig_to_json(self))
    # NOTE: do NOT enable walrus ldw-opt here - it crashes codegen
    # (visitInstLdweights) for 2-byte matmul dtypes; fp16 gets FWL natively.
    _patched = True


# --------------------------------------------------------------------------
# Per-core Bass program
# --------------------------------------------------------------------------


def _build():
    nc = bass.Bass("TRN2", target_bir_lowering=False, debug=False, num_devices=8)

    xq_d = nc.dram_tensor("xq", [128, 4, KC, QB], F16, kind="ExternalInput").ap()
    xk_d = nc.dram_tensor("xk", [128, 4, KC, QB], F16, kind="ExternalInput").ap()
    xv_d = nc.dram_tensor("xv", [128, 4, KC, QB], F16, kind="ExternalInput").ap()
    wq_d = nc.dram_tensor("wq", [128, KC, HG], F16, kind="ExternalInput").ap()
    wk_d = nc.dram_tensor("wk", [128, KC, HG], F16, kind="ExternalInput").ap()
    wv_d = nc.dram_tensor("wv", [128, KC, HG], F16, kind="ExternalInput").ap()
    wo_d = nc.dram_tensor("wo", [128, 2, D], F16, kind="ExternalInput").ap()
    bq_d = nc.dram_tensor("bq", [128, 2], F32, kind="ExternalInput").ap()
    bk_d = nc.dram_tensor("bk", [128, 2], F32, kind="ExternalInput").ap()
    bv_d = nc.dram_tensor("bv", [128, 2 * HG], F32, kind="ExternalInput").ap()
    tri_d = nc.dram_tensor("tri", [128, 256], F16, kind="ExternalInput").ap()
    out_d = nc.dram_tensor("out", [S, D], F16, kind="ExternalOutput").ap()

    from contextlib import ExitStack

    with tile.TileContext(nc) as tc, ExitStack() as ctx:
        consts = ctx.enter_context(tc.tile_pool(name="consts", bufs=1))
        persist = ctx.enter_context(tc.tile_pool(name="persist", bufs=1))
        xin_pool = ctx.enter_context(tc.tile_pool(name="xin", bufs=2))
        et_pool = ctx.enter_context(tc.tile_pool(name="et", bufs=6))
        rcp_pool = ctx.enter_context(tc.tile_pool(name="rcp", bufs=2))
        av_sb_pool = ctx.enter_context(tc.tile_pool(name="avsb", bufs=2))
        osb_pool = ctx.enter_context(tc.tile_pool(name="osb", bufs=2))
        ps_pool = ctx.enter_context(tc.tile_pool(name="ps", bufs=2, space="PSUM"))

        # ---- persistent SBUF state ----
        qp = [persist.tile([128, S], F16, name=f"qp{h}") for h in range(4)]
        kT = persist.tile([128, 2, S], F16, name="kT")
        v_sb = persist.tile([128, 16, 512], F16, name="v")
        attnT = persist.tile([128, 2, S], F16, name="attnT")
        v4 = v_sb.rearrange("p sc (h x) -> p sc h x", x=128)

        # zero the pad half of each per-head q; ones blocks in v (flipped
        # for odd heads so their attn@V output lands on partitions 64:128)
        nc.vector.memset(v4[:, :, 0::2, 64:128], 1.0)
        nc.vector.memset(v4[:, :, 1::2, 0:64], 1.0)
        for h in range(4):
            lo = 64 * (h % 2)
            nc.vector.memset(qp[h][64 - lo : 128 - lo, :], 0.0)

        # ---- const DMAs: small/late-use consts on the GpSimd queue ----
        bq_sb = consts.tile([128, 2], F32, name="bq")
        nc.scalar.dma_start(bq_sb[:], bq_d[:])
        bk_sb = consts.tile([128, 2], F32, name="bk")
        nc.scalar.dma_start(bk_sb[:], bk_d[:])
        bv_sb = consts.tile([128, 2 * HG], F32, name="bv")
        tri_sb = consts.tile([128, 256], F16, name="tri")
        tri2 = tri_sb.rearrange("p (t c) -> p t c", t=2)
        bv4 = bv_sb.rearrange("p (s2 h x) -> p s2 h x", s2=2, h=4)

        # ---- weight + x-input DMAs on the Sync queue, earliest-use first ----
        w_sb = {}
        xin = {"q": [None] * 4, "k": [None] * 4, "v": [None] * 4}

        def load_x(which, dram, qtr, split=False):
            t = xin_pool.tile([128, KC, QB], F16, name=f"x{which}", bufs=3)
            if split:
                nc.sync.dma_start(t[:, 0:4, :], dram[:, qtr, 0:4])
                nc.sync.dma_start(t[:, 4:8, :], dram[:, qtr, 4:8])
            else:
                nc.sync.dma_start(t[:], dram[:, qtr])
            xin[which][qtr] = t

        # startup order interleaves each weight half with the matching x
        # half so the first matmuls' dependencies complete as early as
        # possible on the FIFO DMA queue
        for name, wdram, xdram in (("q", wq_d, xq_d), ("k", wk_d, xk_d), ("v", wv_d, xv_d)):
            t = consts.tile([128, KC, HG], F16, name=f"w{name}")
            w_sb[name] = t
            xt = xin_pool.tile([128, KC, QB], F16, name=f"x{name}", bufs=3)
            nc.sync.dma_start(t[:, 0:4, :], wdram[:, 0:4])
            nc.sync.dma_start(xt[:, 0:4, :], xdram[:, 0, 0:4])
            nc.sync.dma_start(t[:, 4:8, :], wdram[:, 4:8])
            nc.sync.dma_start(xt[:, 4:8, :], xdram[:, 0, 4:8])
            xin[name][0] = xt
        nc.scalar.dma_start(bv_sb[:], bv_d[:])
        nc.scalar.dma_start(tri_sb[:], tri_d[:])
        wo_sb = consts.tile([128, 2, D], F16, name="wo")
        nc.sync.dma_start(wo_sb[:], wo_d[:])
        for qtr in range(1, 4):
            load_x("q", xq_d, qtr)
            load_x("k", xk_d, qtr)
            load_x("v", xv_d, qtr)

        # ---- phase bodies ----

        def qk_frags(qtr):
            # one fragment = one head-pair of the q or k projection for a
            # quarter: 8 accumulating matmuls into an "ot" PSUM bank + the
            # DVE bias evacuation
            frags = []

            def make(which, mi):
                def f():
                    xt = xin[which][qtr]
                    otp = ps_pool.tile([128, 512], F32, name="ot", bufs=2)
                    for kc in range(KC):
                        nc.tensor.matmul(
                            otp[:],
                            w_sb[which][:, kc, 128 * mi : 128 * mi + 128],
                            xt[:, kc, :],
                            start=(kc == 0),
                            stop=(kc == KC - 1),
                        )
                    if which == "q":
                        for par in range(2):
                            h, lo = 2 * mi + par, 64 * par
                            nc.vector.tensor_scalar_add(
                                qp[h][lo : lo + 64, QB * qtr : QB * qtr + QB],
                                otp[lo : lo + 64, :],
                                bq_sb[lo : lo + 64, mi : mi + 1],
                            )
                    else:
                        nc.vector.tensor_scalar_add(
                            kT[:, mi, QB * qtr : QB * qtr + QB],
                            otp[:],
                            bk_sb[:, mi : mi + 1],
                        )
                return f

            for mi in range(2):
                frags.append(make("q", mi))
                frags.append(make("k", mi))
            return frags

        def proj_quarter(qtr):
            for f in qk_frags(qtr):
                f()
            # v projection quarter 0 runs inline (attention block 0 needs it
            # immediately); later quarters are injected as fragments
            if qtr == 0:
                for f in v_frags(0):
                    f()

        def v_frags(qtr):
            # one fragment = one v-projection row-block: 8 accumulating
            # matmuls into half an "ot" PSUM tile + the bias evacuation,
            # with odd heads' dims flipped to cols 64:128
            frags = []

            def make(si):
                def f():
                    xt = xin["v"][qtr]
                    otp = ps_pool.tile([128, 512], F32, name="ot", bufs=2)
                    for kc in range(KC):
                        nc.tensor.matmul(
                            otp[:, 0:HG],
                            xt[:, kc, 128 * si : 128 * si + 128],
                            w_sb["v"][:, kc, :],
                            start=(kc == 0),
                            stop=(kc == KC - 1),
                        )
                    sc = 4 * qtr + si
                    sv4 = otp[:, 0:HG].rearrange("p (h x) -> p h x", h=4)
                    nc.vector.tensor_add(
                        v4[:, sc, 0::2, 0:64], sv4[:, 0::2, :], bv4[:, 0, 0::2, :]
                    )
                    nc.vector.tensor_add(
                        v4[:, sc, 1::2, 64:128], sv4[:, 1::2, :], bv4[:, 0, 1::2, :]
                    )
                return f

            for si in range(4):
                frags.append(make(si))
            return frags

        def attn_block(qb, av_out, frags, rcps=None):
            last = 4 * qb + 3
            n_iters = 2 * (last + 1)
            spacing = n_iters / len(frags) if frags else 0.0
            fi = 0
            it = 0
            for mi in range(2):
                av = ps_pool.tile([128, 1024], F32, name="avacc", bufs=1)
                for kc in range(last + 1):
                    if fi < len(frags) and it >= fi * spacing:
                        frags[fi]()
                        fi += 1
                    it += 1
                    di = kc - 4 * qb
                    w0 = 128 * di if di > 0 else 0
                    sct = ps_pool.tile([128, 1024], F32, name="sc")
                    for par in range(2):
                        h = 2 * mi + par
                        nc.tensor.matmul(
                            sct[:, 512 * par + w0 : 512 * par + 512],
                            kT[:, mi, 128 * kc : 128 * kc + 128],
                            qp[h][:, QB * qb + w0 : QB * qb + QB],
                            start=True,
                            stop=True,
                        )
                    et = et_pool.tile([128, 2, 512], F16, name="et")
                    sv = sct.rearrange("p (two c) -> p two c", two=2)
                    nc.scalar.activation(
                        et[:, :, w0:512],
                        sv[:, :, w0:512],
                        mybir.ActivationFunctionType.Exp,
                        scale=0.125,
                    )
                    if di >= 0:
                        nc.vector.tensor_mul(
                            et[:, :, w0 : w0 + 128], et[:, :, w0 : w0 + 128], tri2[:]
                        )
                    for par in range(2):
                        h = 2 * mi + par
                        nc.tensor.matmul(
                            av[:, 512 * par + w0 : 512 * par + 512],
                            v_sb[:, kc, 128 * h : 128 * h + 128],
                            et[:, par, w0:512],
                            start=(kc == 0),
                            stop=(kc == last),
                            skip_group_check=True,
                        )
                # evacuate raw accumulators to SBUF immediately: frees the
                # PSUM banks without waiting on any normalization math. The
                # very last pass of the kernel skips this (nothing needs its
                # banks) and normalizes straight out of PSUM.
                if qb == 3 and mi == 1:
                    av_out.append(("psum", av))
                else:
                    avs = av_sb_pool.tile([128, 1024], F32, name="avs")
                    nc.vector.tensor_copy(avs[:], av[:])
                    av_out.append(("sbuf", avs))
                # final block: pass 0's normalization runs mid-block (the
                # injected fragments make pass 1 tensor-bound, so ScalarE
                # has slack) leaving only pass 1's chain in the tail
                if rcps is not None and mi == 0:
                    rcps.append(norm_act_one(av_out[0]))
            while fi < len(frags):
                frags[fi]()
                fi += 1

        def norm_act_one(entry):
            # 1/rowsum on ScalarE, deferred to where ScalarE has slack.
            # Even head: dims @0:64, rowsum @64:128 (cols 0:512); odd head
            # flipped (cols 512:1024). 1/x = exp(-ln x); the Exp writes to
            # the dims-aligned partition base so the final DVE muls are
            # same-base (SBUF+SBUF requires it).
            _tag, avs = entry
            rcp = rcp_pool.tile([128, 512], F32, name="rcp")
            nc.scalar.activation(
                avs[64:128, 0:512], avs[64:128, 0:512],
                mybir.ActivationFunctionType.Ln,
            )
            nc.scalar.activation(
                rcp[0:64, :], avs[64:128, 0:512],
                mybir.ActivationFunctionType.Exp, scale=-1.0,
            )
            nc.scalar.activation(
                avs[0:64, 512:1024], avs[0:64, 512:1024],
                mybir.ActivationFunctionType.Ln,
            )
            nc.scalar.activation(
                rcp[64:128, :], avs[0:64, 512:1024],
                mybir.ActivationFunctionType.Exp, scale=-1.0,
            )
            return rcp

        def norm_act(av_out, rcps):
            while len(rcps) < len(av_out):
                rcps.append(norm_act_one(av_out[len(rcps)]))
            return rcps

        def norm_mul(qb, av_out, rcps):
            for mi in range(2):
                (_tag, avs), rcp = av_out[mi], rcps[mi]
                nc.vector.tensor_mul(
                    attnT[0:64, mi, QB * qb : QB * qb + QB],
                    avs[0:64, 0:512],
                    rcp[0:64, :],
                )
                nc.vector.tensor_mul(
                    attnT[64:128, mi, QB * qb : QB * qb + QB],
                    avs[64:128, 512:1024],
                    rcp[64:128, :],
                )

        def outproj_frags(qb, act_split=False, tail=False):
            # one fragment = half an output row-block (one PSUM bank): 2
            # accumulating matmuls + a DVE evacuation. Fragments are injected
            # one-per-kc-iteration into the NEXT attention block, where they
            # give the PE independent work while ScalarE streams exps.
            frags = []
            osb_tiles = {}

            def make(si, nj):
                def f():
                    # tail fragments draw from the (by then idle) score pool
                    # so their PSUM WAR resolves off the exp stream, not the
                    # evacuation-copy stream
                    if tail:
                        otp = ps_pool.tile([128, 1024], F32, name="sc")[:, 0:512]
                    else:
                        otp = ps_pool.tile([128, 512], F32, name="ot", bufs=2)
                    for ci in range(2):
                        nc.tensor.matmul(
                            otp[:],
                            attnT[:, ci, 128 * si : 128 * si + 128],
                            wo_sb[:, ci, 512 * nj : 512 * nj + 512],
                            start=(ci == 0),
                            stop=(ci == 1),
                        )
                    if nj == 0:
                        osb_tiles[si] = osb_pool.tile([128, D], F16, name="osb")
                    osb = osb_tiles[si]
                    if act_split and nj == 0:
                        nc.scalar.activation(
                            osb[:, 0:512], otp[:],
                            mybir.ActivationFunctionType.Copy,
                        )
                    else:
                        nc.vector.tensor_copy(
                            osb[:, 512 * nj : 512 * nj + 512], otp[:]
                        )
                    if nj == 1:
                        nc.gpsimd.dma_start(
                            out_d[128 * si : 128 * si + 128, :], osb[:]
                        )
                return f

            for s4 in range(4):
                si = 4 * qb + s4
                for nj in range(2):
                    frags.append(make(si, nj))
            return frags

        def interleave(a, b):
            out = []
            for i in range(max(len(a), len(b))):
                if i < len(a):
                    out.append(a[i])
                if i < len(b):
                    out.append(b[i])
            return out

        # out-proj fragments are deferred toward the later (larger, more
        # ScalarE-bound) attention blocks: attn(2) absorbs block 0's, attn(3)
        # absorbs blocks 1+2's, and only block 3's remain in the tail.
        pend = None  # (qb, av_out, rcps) awaiting norm_mul
        for qtr in range(4):
            # quarters 2-3 project q/k via fragments injected into the
            # previous attention block; 0-1 run inline (their input DMAs
            # land too late to inject earlier)
            if qtr <= 1:
                proj_quarter(qtr)
            if pend is not None:
                pqb, pav, prcp = pend
                norm_mul(pqb, pav, prcp)
            held = []
            if qtr == 2:
                ofr = outproj_frags(0)
            elif qtr == 3:
                # hold the last 4 fragments back: they fill the PE while the
                # final normalization chain runs on ScalarE/DVE in the tail
                ofr = outproj_frags(1) + outproj_frags(2)[:4]
                held = outproj_frags(2, tail=True)
                held = held[4:]
            else:
                ofr = []
            pfr = v_frags(qtr + 1) if qtr + 1 < 4 else []
            if qtr + 1 in (2, 3):
                pfr = interleave(qk_frags(qtr + 1), pfr)
            av_out = []
            rcps = [] if qtr == 3 else None
            attn_block(qtr, av_out, interleave(ofr, pfr), rcps=rcps)
            for f in held:
                f()
            if rcps is None:
                rcps = []
            norm_act(av_out, rcps)
            pend = (qtr, av_out, rcps)
        pqb, pav, prcp = pend
        norm_mul(pqb, pav, prcp)
        for f in outproj_frags(3, act_split=True, tail=True):
            f()

    return nc


# --------------------------------------------------------------------------
# Host sharding / gathering
# --------------------------------------------------------------------------


def _make_in_maps(Q, K, V, Wq, bq, Wk, bk, Wv, bv, Wo):
    p = np.arange(128)[:, None]
    c = np.arange(128)[None, :]
    tri = np.tile((p <= c).astype(np.float16), (1, 2))

    def xr(Xb):  # [S, D] -> [128, 4, KC, QB] f16
        return np.ascontiguousarray(
            Xb.reshape(4, QB, KC, 128).transpose(3, 0, 2, 1)
        ).astype(np.float16)

    xT = {b: {"q": xr(Q[b]), "k": xr(K[b]), "v": xr(V[b])} for b in range(2)}

    def wr(Wsl):  # [HG, D] -> [128, KC, HG] f16  (w[p,kc,n] = Wsl[n, kc*128+p])
        return np.ascontiguousarray(
            Wsl.T.reshape(KC, 128, HG).transpose(1, 0, 2)
        ).astype(np.float16)

    in_maps = []
    for cix in range(8):
        b, g = divmod(cix, 4)
        sl = slice(HG * g, HG * (g + 1))
        wo_r = np.ascontiguousarray(
            Wo[:, sl].T.reshape(2, 128, D).transpose(1, 0, 2)
        ).astype(np.float16)
        bvb = np.tile(np.broadcast_to(bv[sl].reshape(1, HG), (128, HG)), (1, 2))
        in_maps.append(
            {
                "xq": xT[b]["q"],
                "xk": xT[b]["k"],
                "xv": xT[b]["v"],
                "wq": wr(Wq[sl, :]),
                "wk": wr(Wk[sl, :]),
                "wv": wr(Wv[sl, :]),
                "wo": wo_r,
                "bq": np.ascontiguousarray(bq[sl].reshape(2, 128).T).astype(np.float32),
                "bk": np.ascontiguousarray(bk[sl].reshape(2, 128).T).astype(np.float32),
                "bv": np.ascontiguousarray(bvb).astype(np.float32),
                "tri": tri,
            }
        )
    return in_maps


_nc_cache = None


def kernel(Q, K, V, mask, Wq, bq, Wk, bk, Wv, bv, Wo, bo, **_unused):
    """Full inputs in, full [2, 2048, 1024] float32 output out.

    `mask` is the causal tril mask from setup_inputs(); causality is baked
    into the kernel structure (lower-triangular tiles only + diagonal-tile
    masking), so the tensor itself is not shipped to the device.
    """
    global _nc_cache
    _apply_patches()

    Q, K, V = (np.asarray(x, np.float32) for x in (Q, K, V))
    Wq, Wk, Wv, Wo = (np.asarray(x, np.float32) for x in (Wq, Wk, Wv, Wo))
    bq, bk, bv, bo = (np.asarray(x, np.float32) for x in (bq, bk, bv, bo))

    if _nc_cache is None:
        _nc_cache = _build()
    in_maps = _make_in_maps(Q, K, V, Wq, bq, Wk, bk, Wv, bv, Wo)
    res = bass_utils.run_bass_kernel_spmd(
        _nc_cache, in_maps, core_ids=list(range(8)), trace=False
    )
    out = np.zeros((2, S, D), np.float32)
    for c in range(8):
        out[c // 4] += res.results[c]["out"].astype(np.float32)
    out += bo[None, None, :]
    return out


# revision 61
# speedup vs baseline: 1.0241x; 1.0241x over previous
"""Sharded causal multi-head attention for 8 Trainium2 NeuronCores.

kernel(**inputs) takes the FULL inputs (Q, K, V, mask, Wq, bq, Wk, bk,
Wv, bv, Wo, bo) and returns the FULL [2, 2048, 1024] float32 output.

Sharding (data + head/tensor parallel): core c = 4*b + g handles batch
b in {0,1} and head-group g in {0..3} (4 heads, 256 dims). W_q/W_k/W_v
are column-parallel, W_o row-parallel; the host sums the 4 per-batch
row-parallel partials and adds bo.

Per-core program (Bass/Tile, fp16 compute, fp32 PSUM accumulation),
software-pipelined per S-quarter so projections, attention, and the
output projection all interleave on the PE:

  for quarter in 0..3:
      project q/k/v for this quarter       (PE + DVE bias-evac)
      output-project q-block quarter-1     (PE slack + DVE copy)
      flash attention for q-block quarter  (PE/ACT/DVE pipeline)
  output-project q-block 3

Attention runs in scores^T layout [k, q] per head-pair pass so the exp
covers two heads' score tiles with one ScalarE instruction (PSUM tiles
span 2 banks), amortizing the fixed PSUM/SBUF access overhead. Fully
masked 128-col sub-blocks of diagonal tiles are never computed: score
matmuls, exp, and attn@V all start at the first live column. Causal
masking multiplies a single [128,128] lower-tri tile on the diagonal
block only. Rowsums ride along in the attn@V matmul via a 64-wide ones
block in the stationary operand; odd heads flip [dims|ones] so their
output lands on partitions 64:128 directly (no staging DMA), and the
1/rowsum lives on DVE (reciprocal + cross-partition-base multiply,
which hardware permits when one operand is in PSUM).

Inputs are host-relayouted so every DMA descriptor is an 8KB
contiguous row; x-input DMAs issue from the Sync queue, consts and
output DMAs from the GpSimd queue.
"""

import json
import sys

for _p in ("/opt/trn_rl_repo", "/opt/trn_rl_repo/concourse"):
    if _p not in sys.path:
        sys.path.insert(0, _p)

import numpy as np

import bass_rust
import concourse.bass as bass
import concourse.mybir as mybir
import concourse.tile as tile
from concourse import bass_utils
from concourse.vector_clock import ScopedClock

F32 = mybir.dt.float32
F16 = mybir.dt.float16
S = 2048
D = 1024
HG = 256  # head-group dims (4 heads x 64)
KC = D // 128
QB = 512  # S-quarter

# --------------------------------------------------------------------------
# Environment patches: this container's walrus accepts only ONE sync-wait
# command per instruction, but Tile emits several (and its epilogue drain
# carries one per outstanding proc sem). Split extras onto single-wait NoOps.
# --------------------------------------------------------------------------

_patched = False


def _drain_and_barrier_split(self, tick_clock, wait_clock):
    nc = self.nc
    probe = nc.sync.nop()
    wait_clock.add_sem_waits(probe.ins, ScopedClock({None: tick_clock.global_clock}))
    si = probe.ins.sync_info
    waits = list(si.on_wait) if si is not None and si.on_wait else []
    if len(waits) > 1:
        si.on_wait = [waits[0]]
        for w in waits[1:]:
            nop = nc.sync.nop()
            nop.ins.sync_info = bass_rust.SyncInfo(on_wait=[w], on_update=[])
    nc.sync.drain()
    nc.all_engine_barrier()
    assert self.sems is not None
    popped = nc._tile_sem_poison_stack.pop()
    assert popped is self._sem_poison
    # Skip the per-sem teardown clears + trailing barrier (~7us): the program
    # prologue resets the whole kernel sem range before use, so a fresh
    # execution of this NEFF never observes the dirty values. Keep the pool
    # bookkeeping so tile-system invariants hold during build.
    nc._state.prepend_free_semaphores(
        [s.num for s in self.sems.allocated().values()]
    )
    for poison_set in nc._tile_sem_poison_stack:
        poison_set.update(s.num for s in self.sems.allocated().values())


def _split_waits_json(raw):
    j = json.loads(raw)
    changed = False
    for f in j.get("functions", []):
        for bb in f.get("blocks", []):
            out = []
            for inst in bb.get("instructions", []):
                si = inst.get("sync_info")
                waits = (si or {}).get("on_wait") or []
                if len(waits) > 1:
                    for k, w in enumerate(waits[:-1]):
                        nop = {
                            "engine": inst["engine"],
                            "ins": [],
                            "name": f"{inst['name']}-ws{k}",
                            "opcode": "NoOp",
                            "outs": [],
                            "sync_info": {"on_update": [], "on_wait": [w]},
                        }
                        if "debug" in inst:
                            nop["debug"] = inst["debug"]
                        out.append(nop)
                    si["on_wait"] = [waits[-1]]
                    changed = True
                out.append(inst)
            if changed:
                bb["instructions"] = out
    return json.dumps(j).encode() if changed else raw


def _apply_patches():
    global _patched
    if _patched:
        return
    tile.TileContext._drain_and_barrier = _drain_and_barrier_split
    orig_to_json = bass.Bass.to_json_bytes
    bass.Bass.to_json_bytes = lambda self: _split_waits_json(orig_to_json(self))
    # NOTE: do NOT enable walrus ldw-opt here - it crashes codegen
    # (visitInstLdweights) for 2-byte matmul dtypes; fp16 gets FWL natively.
    _patched = True


# --------------------------------------------------------------------------
# Per-core Bass program
# --------------------------------------------------------------------------


def _build():
    nc = bass.Bass("TRN2", target_bir_lowering=False, debug=False, num_devices=8)

    xq_d = nc.dram_tensor("xq", [128, 4, KC, QB], F16, kind="ExternalInput").ap()
    xk_d = nc.dram_tensor("xk", [128, 4, KC, QB], F16, kind="ExternalInput").ap()
    xv_d = nc.dram_tensor("xv", [128, 4, KC, QB], F16, kind="ExternalInput").ap()
    wq_d = nc.dram_tensor("wq", [128, KC, HG], F16, kind="ExternalInput").ap()
    wk_d = nc.dram_tensor("wk", [128, KC, HG], F16, kind="ExternalInput").ap()
    wv_d = nc.dram_tensor("wv", [128, KC, HG], F16, kind="ExternalInput").ap()
    wo_d = nc.dram_tensor("wo", [128, 2, D], F16, kind="ExternalInput").ap()
    bq_d = nc.dram_tensor("bq", [128, 2], F32, kind="ExternalInput").ap()
    bk_d = nc.dram_tensor("bk", [128, 2], F32, kind="ExternalInput").ap()
    bv_d = nc.dram_tensor("bv", [128, 2 * HG], F32, kind="ExternalInput").ap()
    tri_d = nc.dram_tensor("tri", [128, 256], F16, kind="ExternalInput").ap()
    out_d = nc.dram_tensor("out", [S, D], F16, kind="ExternalOutput").ap()

    from contextlib import ExitStack

    with tile.TileContext(nc) as tc, ExitStack() as ctx:
        consts = ctx.enter_context(tc.tile_pool(name="consts", bufs=1))
        persist = ctx.enter_context(tc.tile_pool(name="persist", bufs=1))
        xin_pool = ctx.enter_context(tc.tile_pool(name="xin", bufs=2))
        et_pool = ctx.enter_context(tc.tile_pool(name="et", bufs=6))
        rcp_pool = ctx.enter_context(tc.tile_pool(name="rcp", bufs=2))
        av_sb_pool = ctx.enter_context(tc.tile_pool(name="avsb", bufs=2))
        osb_pool = ctx.enter_context(tc.tile_pool(name="osb", bufs=2))
        ps_pool = ctx.enter_context(tc.tile_pool(name="ps", bufs=2, space="PSUM"))

        # ---- persistent SBUF state ----
        qp = [persist.tile([128, S], F16, name=f"qp{h}") for h in range(4)]
        kT = persist.tile([128, 2, S], F16, name="kT")
        v_sb = persist.tile([128, 16, 512], F16, name="v")
        attnT = persist.tile([128, 2, S], F16, name="attnT")
        v4 = v_sb.rearrange("p sc (h x) -> p sc h x", x=128)

        # zero the pad half of each per-head q; ones blocks in v (flipped
        # for odd heads so their attn@V output lands on partitions 64:128)
        nc.vector.memset(v4[:, :, 0::2, 64:128], 1.0)
        nc.vector.memset(v4[:, :, 1::2, 0:64], 1.0)
        for h in range(4):
            lo = 64 * (h % 2)
            nc.vector.memset(qp[h][64 - lo : 128 - lo, :], 0.0)

        # ---- const DMAs: small/late-use consts on the GpSimd queue ----
        bq_sb = consts.tile([128, 2], F32, name="bq")
        nc.scalar.dma_start(bq_sb[:], bq_d[:])
        bk_sb = consts.tile([128, 2], F32, name="bk")
        nc.scalar.dma_start(bk_sb[:], bk_d[:])
        bv_sb = consts.tile([128, 2 * HG], F32, name="bv")
        tri_sb = consts.tile([128, 256], F16, name="tri")
        tri2 = tri_sb.rearrange("p (t c) -> p t c", t=2)
        bv4 = bv_sb.rearrange("p (s2 h x) -> p s2 h x", s2=2, h=4)

        # ---- weight + x-input DMAs on the Sync queue, earliest-use first ----
        w_sb = {}
        xin = {"q": [None] * 4, "k": [None] * 4, "v": [None] * 4}

        def load_x(which, dram, qtr, split=False):
            t = xin_pool.tile([128, KC, QB], F16, name=f"x{which}", bufs=3)
            if split:
                nc.sync.dma_start(t[:, 0:4, :], dram[:, qtr, 0:4])
                nc.sync.dma_start(t[:, 4:8, :], dram[:, qtr, 4:8])
            else:
                nc.sync.dma_start(t[:], dram[:, qtr])
            xin[which][qtr] = t

        # startup order interleaves each weight half with the matching x
        # half so the first matmuls' dependencies complete as early as
        # possible on the FIFO DMA queue
        for name, wdram, xdram in (("q", wq_d, xq_d), ("k", wk_d, xk_d), ("v", wv_d, xv_d)):
            t = consts.tile([128, KC, HG], F16, name=f"w{name}")
            w_sb[name] = t
            xt = xin_pool.tile([128, KC, QB], F16, name=f"x{name}", bufs=3)
            nc.sync.dma_start(t[:, 0:4, :], wdram[:, 0:4])
            nc.sync.dma_start(xt[:, 0:4, :], xdram[:, 0, 0:4])
            nc.sync.dma_start(t[:, 4:8, :], wdram[:, 4:8])
            nc.sync.dma_start(xt[:, 4:8, :], xdram[:, 0, 4:8])
            xin[name][0] = xt
        nc.scalar.dma_start(bv_sb[:], bv_d[:])
        nc.scalar.dma_start(tri_sb[:], tri_d[:])
        wo_sb = consts.tile([128, 2, D], F16, name="wo")
        nc.sync.dma_start(wo_sb[:], wo_d[:])
        for qtr in range(1, 4):
            load_x("q", xq_d, qtr)
            load_x("k", xk_d, qtr)
            load_x("v", xv_d, qtr)

        # ---- phase bodies ----

        def qk_frags(qtr):
            # one fragment = one head-pair of the q or k projection for a
            # quarter: 8 accumulating matmuls into an "ot" PSUM bank + the
            # DVE bias evacuation
            frags = []

            def make(which, mi):
                def f():
                    xt = xin[which][qtr]
                    otp = ps_pool.tile([128, 512], F32, name="ot", bufs=2)
                    for kc in range(KC):
                        nc.tensor.matmul(
                            otp[:],
                            w_sb[which][:, kc, 128 * mi : 128 * mi + 128],
                            xt[:, kc, :],
                            start=(kc == 0),
                            stop=(kc == KC - 1),
                        )
                    if which == "q":
                        for par in range(2):
                            h, lo = 2 * mi + par, 64 * par
                            nc.vector.tensor_scalar_add(
                                qp[h][lo : lo + 64, QB * qtr : QB * qtr + QB],
                                otp[lo : lo + 64, :],
                                bq_sb[lo : lo + 64, mi : mi + 1],
                            )
                    else:
                        nc.vector.tensor_scalar_add(
                            kT[:, mi, QB * qtr : QB * qtr + QB],
                            otp[:],
                            bk_sb[:, mi : mi + 1],
                        )
                return f

            for mi in range(2):
                frags.append(make("q", mi))
                frags.append(make("k", mi))
            return frags

        def proj_quarter(qtr):
            for f in qk_frags(qtr):
                f()
            # v projection quarter 0 runs inline (attention block 0 needs it
            # immediately); later quarters are injected as fragments
            if qtr == 0:
                for f in v_frags(0):
                    f()

        def v_frags(qtr):
            # one fragment = one v-projection row-block: 8 accumulating
            # matmuls into half an "ot" PSUM tile + the bias evacuation,
            # with odd heads' dims flipped to cols 64:128
            frags = []

            def make(si):
                def f():
                    xt = xin["v"][qtr]
                    otp = ps_pool.tile([128, 512], F32, name="ot", bufs=2)
                    for kc in range(KC):
                        nc.tensor.matmul(
                            otp[:, 0:HG],
                            xt[:, kc, 128 * si : 128 * si + 128],
                            w_sb["v"][:, kc, :],
                            start=(kc == 0),
                            stop=(kc == KC - 1),
                        )
                    sc = 4 * qtr + si
                    sv4 = otp[:, 0:HG].rearrange("p (h x) -> p h x", h=4)
                    nc.vector.tensor_add(
                        v4[:, sc, 0::2, 0:64], sv4[:, 0::2, :], bv4[:, 0, 0::2, :]
                    )
                    nc.vector.tensor_add(
                        v4[:, sc, 1::2, 64:128], sv4[:, 1::2, :], bv4[:, 0, 1::2, :]
                    )
                return f

            for si in range(4):
                frags.append(make(si))
            return frags

        def attn_block(qb, av_out, frags, rcps=None):
            last = 4 * qb + 3
            n_iters = 2 * (last + 1)
            spacing = n_iters / len(frags) if frags else 0.0
            fi = 0
            it = 0
            for mi in range(2):
                av = ps_pool.tile([128, 1024], F32, name="avacc", bufs=1)
                for kc in range(last + 1):
                    if fi < len(frags) and it >= fi * spacing:
                        frags[fi]()
                        fi += 1
                    it += 1
                    di = kc - 4 * qb
                    w0 = 128 * di if di > 0 else 0
                    sct = ps_pool.tile([128, 1024], F32, name="sc")
                    for par in range(2):
                        h = 2 * mi + par
                        nc.tensor.matmul(
                            sct[:, 512 * par + w0 : 512 * par + 512],
                            kT[:, mi, 128 * kc : 128 * kc + 128],
                            qp[h][:, QB * qb + w0 : QB * qb + QB],
                            start=True,
                            stop=True,
                        )
                    et = et_pool.tile([128, 2, 512], F16, name="et")
                    sv = sct.rearrange("p (two c) -> p two c", two=2)
                    nc.scalar.activation(
                        et[:, :, w0:512],
                        sv[:, :, w0:512],
                        mybir.ActivationFunctionType.Exp,
                        scale=0.125,
                    )
                    if di >= 0:
                        nc.vector.tensor_mul(
                            et[:, :, w0 : w0 + 128], et[:, :, w0 : w0 + 128], tri2[:]
                        )
                    for par in range(2):
                        h = 2 * mi + par
                        nc.tensor.matmul(
                            av[:, 512 * par + w0 : 512 * par + 512],
                            v_sb[:, kc, 128 * h : 128 * h + 128],
                            et[:, par, w0:512],
                            start=(kc == 0),
                            stop=(kc == last),
                            skip_group_check=True,
                        )
                # evacuate raw accumulators to SBUF immediately: frees the
                # PSUM banks without waiting on any normalization math. The
                # very last pass of the kernel skips this (nothing needs its
                # banks) and normalizes straight out of PSUM.
                if qb == 3 and mi == 1:
                    av_out.append(("psum", av))
                else:
                    avs = av_sb_pool.tile([128, 1024], F32, name="avs")
                    nc.vector.tensor_copy(avs[:], av[:])
                    av_out.append(("sbuf", avs))
                # final block: pass 0's normalization runs mid-block (the
                # injected fragments make pass 1 tensor-bound, so ScalarE
                # has slack) leaving only pass 1's chain in the tail
                if rcps is not None and mi == 0:
                    rcps.append(norm_act_one(av_out[0]))
            while fi < len(frags):
                frags[fi]()
                fi += 1

        def norm_act_one(entry):
            # 1/rowsum on ScalarE, deferred to where ScalarE has slack.
            # Even head: dims @0:64, rowsum @64:128 (cols 0:512); odd head
            # flipped (cols 512:1024). 1/x = exp(-ln x); the Exp writes to
            # the dims-aligned partition base so the final DVE muls are
            # same-base (SBUF+SBUF requires it).
            _tag, avs = entry
            rcp = rcp_pool.tile([128, 512], F32, name="rcp")
            nc.scalar.activation(
                avs[64:128, 0:512], avs[64:128, 0:512],
                mybir.ActivationFunctionType.Ln,
            )
            nc.scalar.activation(
                rcp[0:64, :], avs[64:128, 0:512],
                mybir.ActivationFunctionType.Exp, scale=-1.0,
            )
            nc.scalar.activation(
                avs[0:64, 512:1024], avs[0:64, 512:1024],
                mybir.ActivationFunctionType.Ln,
            )
            nc.scalar.activation(
                rcp[64:128, :], avs[0:64, 512:1024],
                mybir.ActivationFunctionType.Exp, scale=-1.0,
            )
            return rcp

        def norm_act(av_out, rcps):
            while len(rcps) < len(av_out):
                rcps.append(norm_act_one(av_out[len(rcps)]))
            return rcps

        def norm_mul(qb, av_out, rcps):
            for mi in range(2):
                (_tag, avs), rcp = av_out[mi], rcps[mi]
                nc.vector.tensor_mul(
                    attnT[0:64, mi, QB * qb : QB * qb + QB],
                    avs[0:64, 0:512],
                    rcp[0:64, :],
                )
                nc.vector.tensor_mul(
                    attnT[64:128, mi, QB * qb : QB * qb + QB],
                    avs[64:128, 512:1024],
                    rcp[64:128, :],
                )

        def outproj_frags(qb, act_split=False, tail=False):
            # one fragment = half an output row-block (one PSUM bank): 2
            # accumulating matmuls + a DVE evacuation. Fragments are injected
            # one-per-kc-iteration into the NEXT attention block, where they
            # give the PE independent work while ScalarE streams exps.
            frags = []
            osb_tiles = {}

            def make(si, nj):
                def f():
                    # tail fragments draw from the (by then idle) score pool
                    # so their PSUM WAR resolves off the exp stream, not the
                    # evacuation-copy stream
                    if tail:
                        otp = ps_pool.tile([128, 1024], F32, name="sc")[:, 0:512]
                    else:
                        otp = ps_pool.tile([128, 512], F32, name="ot", bufs=2)
                    for ci in range(2):
                        nc.tensor.matmul(
                            otp[:],
                            attnT[:, ci, 128 * si : 128 * si + 128],
                            wo_sb[:, ci, 512 * nj : 512 * nj + 512],
                            start=(ci == 0),
                            stop=(ci == 1),
                        )
                    if nj == 0:
                        osb_tiles[si] = osb_pool.tile([128, D], F16, name="osb")
                    osb = osb_tiles[si]
                    if act_split and nj == 0:
                        nc.scalar.activation(
                            osb[:, 0:512], otp[:],
                            mybir.ActivationFunctionType.Copy,
                        )
                    else:
                        nc.vector.tensor_copy(
                            osb[:, 512 * nj : 512 * nj + 512], otp[:]
                        )
                    if nj == 1:
                        nc.gpsimd.dma_start(
                            out_d[128 * si : 128 * si + 128, :], osb[:]
                        )
                return f

            for s4 in range(4):
                si = 4 * qb + s4
                for nj in range(2):
                    frags.append(make(si, nj))
            return frags

        def interleave(a, b):
            out = []
            for i in range(max(len(a), len(b))):
                if i < len(a):
                    out.append(a[i])
                if i < len(b):
                    out.append(b[i])
            return out

        # out-proj fragments are deferred toward the later (larger, more
        # ScalarE-bound) attention blocks: attn(2) absorbs block 0's, attn(3)
        # absorbs blocks 1+2's, and only block 3's remain in the tail.
        pend = None  # (qb, av_out, rcps) awaiting norm_mul
        for qtr in range(4):
            proj_quarter(qtr)
            if pend is not None:
                pqb, pav, prcp = pend
                norm_mul(pqb, pav, prcp)
            held = []
            if qtr == 2:
                ofr = outproj_frags(0)
            elif qtr == 3:
                # hold the last 4 fragments back: they fill the PE while the
                # final normalization chain runs on ScalarE/DVE in the tail
                ofr = outproj_frags(1) + outproj_frags(2)[:4]
                held = outproj_frags(2, tail=True)
                held = held[4:]
            else:
                ofr = []
            pfr = v_frags(qtr + 1) if qtr + 1 < 4 else []
            av_out = []
            rcps = [] if qtr == 3 else None
            attn_block(qtr, av_out, interleave(ofr, pfr), rcps=rcps)
            for f in held:
                f()
            if rcps is None:
                rcps = []
            norm_act(av_out, rcps)
            pend = (qtr, av_out, rcps)
        pqb, pav, prcp = pend
        norm_mul(pqb, pav, prcp)
        for f in outproj_frags(3, act_split=True, tail=True):
            f()

    return nc


# --------------------------------------------------------------------------
# Host sharding / gathering
# --------------------------------------------------------------------------


def _make_in_maps(Q, K, V, Wq, bq, Wk, bk, Wv, bv, Wo):
    p = np.arange(128)[:, None]
    c = np.arange(128)[None, :]
    tri = np.tile((p <= c).astype(np.float16), (1, 2))

    def xr(Xb):  # [S, D] -> [128, 4, KC, QB] f16
        return np.ascontiguousarray(
            Xb.reshape(4, QB, KC, 128).transpose(3, 0, 2, 1)
        ).astype(np.float16)

    xT = {b: {"q": xr(Q[b]), "k": xr(K[b]), "v": xr(V[b])} for b in range(2)}

    def wr(Wsl):  # [HG, D] -> [128, KC, HG] f16  (w[p,kc,n] = Wsl[n, kc*128+p])
        return np.ascontiguousarray(
            Wsl.T.reshape(KC, 128, HG).transpose(1, 0, 2)
        ).astype(np.float16)

    in_maps = []
    for cix in range(8):
        b, g = divmod(cix, 4)
        sl = slice(HG * g, HG * (g + 1))
        wo_r = np.ascontiguousarray(
            Wo[:, sl].T.reshape(2, 128, D).transpose(1, 0, 2)
        ).astype(np.float16)
        bvb = np.tile(np.broadcast_to(bv[sl].reshape(1, HG), (128, HG)), (1, 2))
        in_maps.append(
            {
                "xq": xT[b]["q"],
                "xk": xT[b]["k"],
                "xv": xT[b]["v"],
                "wq": wr(Wq[sl, :]),
                "wk": wr(Wk[sl, :]),
                "wv": wr(Wv[sl, :]),
                "wo": wo_r,
                "bq": np.ascontiguousarray(bq[sl].reshape(2, 128).T).astype(np.float32),
                "bk": np.ascontiguousarray(bk[sl].reshape(2, 128).T).astype(np.float32),
                "bv": np.ascontiguousarray(bvb).astype(np.float32),
                "tri": tri,
            }
        )
    return in_maps


_nc_cache = None


def kernel(Q, K, V, mask, Wq, bq, Wk, bk, Wv, bv, Wo, bo, **_unused):
    """Full inputs in, full [2, 2048, 1024] float32 output out.

    `mask` is the causal tril mask from setup_inputs(); causality is baked
    into the kernel structure (lower-triangular tiles only + diagonal-tile
    masking), so the tensor itself is not shipped to the device.
    """
    global _nc_cache
    _apply_patches()

    Q, K, V = (np.asarray(x, np.float32) for x in (Q, K, V))
    Wq, Wk, Wv, Wo = (np.asarray(x, np.float32) for x in (Wq, Wk, Wv, Wo))
    bq, bk, bv, bo = (np.asarray(x, np.float32) for x in (bq, bk, bv, bo))

    if _nc_cache is None:
        _nc_cache = _build()
    in_maps = _make_in_maps(Q, K, V, Wq, bq, Wk, bk, Wv, bv, Wo)
    res = bass_utils.run_bass_kernel_spmd(
        _nc_cache, in_maps, core_ids=list(range(8)), trace=False
    )
    out = np.zeros((2, S, D), np.float32)
    for c in range(8):
        out[c // 4] += res.results[c]["out"].astype(np.float32)
    out += bo[None, None, :]
    return out
